# revision 5
# baseline (speedup 1.0000x reference)
"""Trainium2 Bass kernel for HarmonyTransformer (relative-position attention block).

Sharding (fast path): 2-D mesh over 8 NeuronCores = 4 head-groups x 2
batch-groups. Core c handles batches [bg*4, bg*4+4) (bg = c//4) and heads
{2*hg, 2*hg+1} (hg = c%4) for the FULL sequence. q/k/v/r projections are
sliced 8-fold (vs replicated in a pure sequence shard); the only cross-core
step is a bf16 ReduceScatter (groups of 4 cores sharing a batch group) that
sums the per-head-group output-projection partials and hands each core one
sequence quarter, on which it applies residual + LayerNorm locally.

Fast path requires the Transformer-XL Toeplitz structure
pos_emb[q,k] = table[k-q+511] (reference.setup_inputs always produces it):
r = table @ Wr_pair.T is computed once per core ([128 dh, 1023]); bd scores
are computed in table space per 128-query chunk ([128, 640] windows) and
re-read per-query-shifted through a DRAM bounce using a pitch-639 strided
view (row r starts 1 element earlier than row r-1).

Math identities used (exact):
  - bk and br shift every score in a softmax row by a constant -> dropped.
  - bv passes through attention and Wo linearly: bo2 = bo + Wo @ bv, and
    bo2 is pre-added to the residual on the host.
  - bq folded into per-head biases: biasU = bq + u_bias, biasV = bq + v_bias.
  - softmax normalization deferred: unnormalized exp scores are transposed
    on the PE with rhs = diag(1/rowsum) instead of the identity, making the
    normalization free.

Device compute is fp16 multiplies with fp32 PSUM accumulation. A structure
spot-check on the host picks the fast path; arbitrary pos_emb falls back to
the original sequence-sharded kernel (_build).
"""

import numpy as np

import concourse.bass as bass
import concourse.bacc as bacc
import concourse.mybir as mybir
import concourse.tile as tile
from concourse.masks import make_identity
from concourse.bass_utils import run_bass_kernel_spmd

B, S, D, H, DH = 8, 512, 512, 8, 64
NCORES = 8
Q = S // NCORES            # fallback kernel: 64 queries per core
NCH = 4                    # 128-partition chunks of D
F32 = mybir.dt.float32
F16 = mybir.dt.float16
BF16 = mybir.dt.bfloat16
LN_EPS = 1e-5
TW = 575                   # fallback: per-core table-slice rows
TWP = 576
BDP = 640

# fast path (2-D shard) constants
BL = 4                     # local batches per core
JW = 1024                  # padded table width (1023 rows + 1 pad)
PITCH = 640                # bd_raw chunk window width / DRAM row pitch

_CACHE = {}
last_result = None


def _build_2d():
    nc = bacc.Bacc()

    # ---- DRAM I/O (per-core shapes; data differs per core) ----
    qt_d = nc.dram_tensor("qt", [128, NCH, BL, S], F16, kind="ExternalInput")
    kt_d = nc.dram_tensor("kt", [128, NCH, BL, S], F16, kind="ExternalInput")
    vt_d = nc.dram_tensor("vt", [128, NCH, BL, S], F16, kind="ExternalInput")
    pet_d = nc.dram_tensor("pet", [128, NCH, JW], F16, kind="ExternalInput")
    wq_d = nc.dram_tensor("wqp", [128, NCH, 128], F16, kind="ExternalInput")
    wk_d = nc.dram_tensor("wkp", [128, NCH, 128], F16, kind="ExternalInput")
    wv_d = nc.dram_tensor("wvp", [128, NCH, 128], F16, kind="ExternalInput")
    wr_d = nc.dram_tensor("wrp", [128, NCH, 128], F16, kind="ExternalInput")
    wo_d = nc.dram_tensor("wop", [128, D], F16, kind="ExternalInput")
    bu_d = nc.dram_tensor("bu", [128, 1], F32, kind="ExternalInput")
    bv_d = nc.dram_tensor("bv2", [128, 1], F32, kind="ExternalInput")
    qn_d = nc.dram_tensor("qn", [BL, 128, D], F32, kind="ExternalInput")
    lg_d = nc.dram_tensor("lng", [1, D], F32, kind="ExternalInput")
    lb_d = nc.dram_tensor("lnb", [1, D], F32, kind="ExternalInput")
    out_d = nc.dram_tensor("out", [BL, 128, D], F32, kind="ExternalOutput")
    bdr_bs = [nc.dram_tensor(f"bdr{b}", [2 * NCH * 128 * PITCH], F16)
              for b in range(BL)]

    with tile.TileContext(nc) as tc:
        with tc.tile_pool(name="consts", bufs=1) as consts, \
             tc.tile_pool(name="dpool", bufs=1, space="DRAM") as dpool:
            rs_ins = [dpool.tile([NCH, 128, D], BF16, tag=f"rsin{b}",
                                 name=f"rsin{b}") for b in range(BL)]
            rs_outs = [dpool.tile([128, D], BF16, tag=f"rsout{b}",
                                  name=f"rsout{b}") for b in range(BL)]

            ident = consts.tile([128, 128], F16)
            make_identity(nc, ident)

            wo_sb = consts.tile([128, D], F16, tag="wo")
            nc.sync.dma_start(out=wo_sb, in_=wo_d[:])
            bu_ld = consts.tile([128, 1], F32, tag="buld")
            nc.sync.dma_start(out=bu_ld, in_=bu_d[:])
            bv_ld = consts.tile([128, 1], F32, tag="bvld")
            nc.sync.dma_start(out=bv_ld, in_=bv_d[:])
            bu_sb = consts.tile([128, 1], F32, tag="bu")
            nc.vector.tensor_copy(out=bu_sb, in_=bu_ld)
            bv_sb = consts.tile([128, 1], F32, tag="bv")
            nc.vector.tensor_copy(out=bv_sb, in_=bv_ld)
            lg_sb = consts.tile([128, D], F32, tag="lg")
            nc.sync.dma_start(out=lg_sb, in_=lg_d[:].to_broadcast((128, D)))
            lb_sb = consts.tile([128, D], F32, tag="lb")
            nc.sync.dma_start(out=lb_sb, in_=lb_d[:].to_broadcast((128, D)))
            eps_sb = consts.tile([128, 1], F32, tag="eps")
            nc.vector.memset(eps_sb, LN_EPS)

            rt_sb = consts.tile([128, JW], F16, tag="rt")        # [dh-pair, j]
            qu_sb = consts.tile([128, BL, S], F16, tag="qu")     # [dh-pair, b, q]
            qv_sb = consts.tile([128, BL, S], F16, tag="qv")
            kh_sb = consts.tile([128, BL, S], F16, tag="kh")     # [dh-pair, b, k]
            vh_sb = consts.tile([128, BL, NCH, 128], F16, tag="vh")  # [k_in, b, kc, dh]

            # ---------------- Phase A: projections ----------------
            with tc.tile_pool(name="pha", bufs=1) as pha, \
                 tc.tile_pool(name="psA", bufs=3, space="PSUM") as psA:
                wr_sb = pha.tile([128, NCH, 128], F16, tag="wr")
                nc.sync.dma_start(out=wr_sb, in_=wr_d[:])
                pet_sb = pha.tile([128, NCH, JW], F16, tag="pet")
                nc.sync.dma_start(out=pet_sb, in_=pet_d[:])
                wq_sb = pha.tile([128, NCH, 128], F16, tag="wq")
                nc.sync.dma_start(out=wq_sb, in_=wq_d[:])
                wk_sb = pha.tile([128, NCH, 128], F16, tag="wk")
                nc.sync.dma_start(out=wk_sb, in_=wk_d[:])
                wv_sb = pha.tile([128, NCH, 128], F16, tag="wv")
                nc.sync.dma_start(out=wv_sb, in_=wv_d[:])
                qt_sb = pha.tile([128, NCH, BL, S], F16, tag="qt")
                nc.sync.dma_start(out=qt_sb, in_=qt_d[:])
                kt_sb = pha.tile([128, NCH, BL, S], F16, tag="kt")
                nc.sync.dma_start(out=kt_sb, in_=kt_d[:])
                vt_sb = pha.tile([128, NCH, BL, S], F16, tag="vt")
                nc.sync.dma_start(out=vt_sb, in_=vt_d[:])

                # r-table projection: rt = Wr_pair.T-proj of table  [128 dh, JW]
                for jh in range(2):
                    ps_r = psA.tile([128, 512], F32, tag="psA")
                    for c in range(NCH):
                        nc.tensor.matmul(ps_r, wr_sb[:, c, :],
                                         pet_sb[:, c, jh * 512:(jh + 1) * 512],
                                         start=(c == 0), stop=(c == NCH - 1))
                    if jh == 0:
                        nc.vector.tensor_copy(out=rt_sb[:, 0:512], in_=ps_r)
                    else:
                        nc.scalar.copy(out=rt_sb[:, 512:JW], in_=ps_r)

                # q projection (+ biases) -> qu / qv
                for b in range(BL):
                    ps_q = psA.tile([128, 512], F32, tag="psA")
                    for c in range(NCH):
                        nc.tensor.matmul(ps_q, wq_sb[:, c, :], qt_sb[:, c, b, :],
                                         start=(c == 0), stop=(c == NCH - 1))
                    nc.scalar.activation(out=qu_sb[:, b, :], in_=ps_q,
                                         func=mybir.ActivationFunctionType.Identity,
                                         bias=bu_sb)
                    nc.scalar.activation(out=qv_sb[:, b, :], in_=ps_q,
                                         func=mybir.ActivationFunctionType.Identity,
                                         bias=bv_sb)

                # k projection
                for b in range(BL):
                    ps_k = psA.tile([128, 512], F32, tag="psA")
                    for c in range(NCH):
                        nc.tensor.matmul(ps_k, wk_sb[:, c, :], kt_sb[:, c, b, :],
                                         start=(c == 0), stop=(c == NCH - 1))
                    nc.vector.tensor_copy(out=kh_sb[:, b, :], in_=ps_k)

                # v projection in [dh, k] layout, then DMA-transpose to
                # vh [k_in, b, kc, dh-pair]
                for b in range(BL):
                    ps_v = psA.tile([128, 512], F32, tag="psA")
                    for c in range(NCH):
                        nc.tensor.matmul(ps_v, wv_sb[:, c, :], vt_sb[:, c, b, :],
                                         start=(c == 0), stop=(c == NCH - 1))
                    vht = pha.tile([128, 512], F16, tag=f"vht{b % 2}")
                    if b % 2 == 0:
                        nc.vector.tensor_copy(out=vht, in_=ps_v)
                    else:
                        nc.scalar.copy(out=vht, in_=ps_v)
                    nc.sync.dma_start_transpose(out=vh_sb[:, b], in_=vht[:])

            # ------- Phases B-D fused per batch: bd_raw -> scores/softmax ->
            # ------- PV -> out-proj -> chunked ReduceScatter ----------------
            with tc.tile_pool(name="phb", bufs=3) as phb, \
                 tc.tile_pool(name="phc", bufs=4) as phc, \
                 tc.tile_pool(name="phc2", bufs=3) as phc2, \
                 tc.tile_pool(name="phd", bufs=3) as phd, \
                 tc.tile_pool(name="attnp", bufs=2) as attnp, \
                 tc.tile_pool(name="ps512", bufs=3, space="PSUM") as ps512, \
                 tc.tile_pool(name="ps128", bufs=2, space="PSUM") as ps128, \
                 tc.tile_pool(name="psD", bufs=2, space="PSUM") as psD, \
                 tc.tile_pool(name="psO", bufs=1, space="PSUM") as psO:
                for b in range(BL):
                    bdr_w = bdr_bs[b][:].rearrange("(h c q j) -> h c q j",
                                                   h=2, c=NCH, q=128, j=PITCH)
                    # bd_raw chunks -> DRAM bounce
                    i = 0
                    for qc in range(NCH):
                        jc = 384 - 128 * qc
                        for h in range(2):
                            lhs = qv_sb[h * 64:(h + 1) * 64, b, qc * 128:(qc + 1) * 128]
                            ps1 = ps512.tile([128, 512], F32, tag="ps512")
                            nc.tensor.matmul(ps1, lhs,
                                             rt_sb[h * 64:(h + 1) * 64, jc:jc + 512],
                                             start=True, stop=True)
                            ps2 = ps128.tile([128, 128], F32, tag="ps128")
                            nc.tensor.matmul(ps2, lhs,
                                             rt_sb[h * 64:(h + 1) * 64, jc + 512:jc + PITCH],
                                             start=True, stop=True)
                            bd_st = phb.tile([128, PITCH], F16, tag="bdst")
                            if i % 2 == 0:
                                nc.vector.tensor_copy(out=bd_st[:, 0:512], in_=ps1)
                                nc.scalar.copy(out=bd_st[:, 512:PITCH], in_=ps2)
                            else:
                                nc.scalar.copy(out=bd_st[:, 0:512], in_=ps1)
                                nc.vector.tensor_copy(out=bd_st[:, 512:PITCH], in_=ps2)
                            nc.sync.dma_start(out=bdr_w[h, qc], in_=bd_st)
                            i += 1

                    # scores + softmax + transposed attn
                    attn = attnp.tile([128, NCH, 2, S], F16, tag="attn")  # [k_in, kc, h, q]
                    for qc in range(NCH):
                        for h in range(2):
                            base = ((h * NCH + qc) * 128) * PITCH + 127
                            bd_sh = phc.tile([128, 512], F16, tag="bdsh")
                            nc.sync.dma_start(
                                out=bd_sh,
                                in_=bdr_bs[b][base:base + 128 * (PITCH - 1)]
                                .rearrange("(q j) -> q j", j=PITCH - 1)[:, 0:512])
                            ps_sc = ps512.tile([128, 512], F32, tag="ps512")
                            nc.tensor.matmul(ps_sc,
                                             qu_sb[h * 64:(h + 1) * 64, b, qc * 128:(qc + 1) * 128],
                                             kh_sb[h * 64:(h + 1) * 64, b, :],
                                             start=True, stop=False)
                            nc.tensor.matmul(ps_sc, ident, bd_sh,
                                             start=False, stop=True)
                            # softmax over k (free axis); scale 1/sqrt(DH)=0.125.
                            # No max subtraction: |score|/8 is far inside exp range.
                            p16 = phc2.tile([128, 512], F16, tag="p16")
                            rsum = phc2.tile([128, 1], F32, tag="rsum")
                            nc.scalar.activation(out=p16, in_=ps_sc,
                                                 func=mybir.ActivationFunctionType.Exp,
                                                 scale=0.125, accum_out=rsum)
                            rc = phc2.tile([128, 1], F32, tag="rc")
                            nc.vector.reciprocal(out=rc, in_=rsum)
                            p16n = phc2.tile([128, 512], F16, tag="p16n")
                            nc.vector.tensor_scalar_mul(out=p16n, in0=p16, scalar1=rc)
                            nc.sync.dma_start_transpose(
                                out=attn[:, :, h, qc * 128:(qc + 1) * 128],
                                in_=p16n[:])

                    # PV: [dh-pair, q] accumulated over k chunks
                    ps_pv = psD.tile([128, 512], F32, tag="pspv")
                    for h in range(2):
                        for kc in range(NCH):
                            nc.tensor.matmul(ps_pv[h * 64:(h + 1) * 64, :],
                                             vh_sb[:, b, kc, h * 64:(h + 1) * 64],
                                             attn[:, kc, h, :],
                                             start=(kc == 0), stop=(kc == NCH - 1))
                    aot = phd.tile([128, 512], F16, tag="aot")
                    if b % 2 == 0:
                        nc.vector.tensor_copy(out=aot, in_=ps_pv)
                    else:
                        nc.scalar.copy(out=aot, in_=ps_pv)
                    # out-projection partials -> bf16 -> per-batch ReduceScatter
                    for qc in range(NCH):
                        ps_o = psO.tile([128, 512], F32, tag="pso")
                        nc.tensor.matmul(ps_o, aot[:, qc * 128:(qc + 1) * 128], wo_sb,
                                         start=True, stop=True)
                        ro = phd.tile([128, 512], BF16, tag="ro")
                        if (b + qc) % 2 == 0:
                            nc.scalar.copy(out=ro, in_=ps_o)
                        else:
                            nc.vector.tensor_copy(out=ro, in_=ps_o)
                        nc.sync.dma_start(out=rs_ins[b][qc], in_=ro)
                    nc.gpsimd.collective_compute(
                        "ReduceScatter", mybir.AluOpType.add,
                        replica_groups=[[0, 1, 2, 3], [4, 5, 6, 7]],
                        ins=[rs_ins[b].opt()], outs=[rs_outs[b].opt()])

            # ---------------- Epilogue: residual + LayerNorm ----------------
            with tc.tile_pool(name="phe", bufs=3) as phd:
                for b in range(BL):
                    rr = phd.tile([128, D], BF16, tag="rr")
                    nc.sync.dma_start(out=rr, in_=rs_outs[b][:])
                    qn_b = phd.tile([128, D], F32, tag="qnb")
                    nc.sync.dma_start(out=qn_b, in_=qn_d[b])
                    o1 = phd.tile([128, D], F32, tag="o1")
                    nc.vector.tensor_add(out=o1, in0=rr, in1=qn_b)
                    st6 = phd.tile([128, nc.vector.BN_STATS_DIM], F32, tag="st6")
                    nc.vector.bn_stats(out=st6, in_=o1)
                    mv = phd.tile([128, nc.vector.BN_AGGR_DIM], F32, tag="mv")
                    nc.vector.bn_aggr(out=mv, in_=st6)
                    sd = phd.tile([128, 1], F32, tag="sd")
                    nc.scalar.activation(out=sd, in_=mv[:, 1:2],
                                         func=mybir.ActivationFunctionType.Sqrt,
                                         bias=eps_sb, scale=1.0)
                    rstd = phd.tile([128, 1], F32, tag="rstd")
                    nc.vector.reciprocal(out=rstd, in_=sd)
                    mr = phd.tile([128, 1], F32, tag="mr")
                    nc.vector.tensor_mul(out=mr, in0=mv[:, 0:1], in1=rstd)
                    nmr = phd.tile([128, 1], F32, tag="nmr")
                    nc.vector.tensor_scalar_mul(out=nmr, in0=mr, scalar1=-1.0)
                    o3 = phd.tile([128, D], F32, tag="o3")
                    nc.scalar.activation(out=o3, in_=o1,
                                         func=mybir.ActivationFunctionType.Identity,
                                         bias=nmr, scale=rstd)
                    o4 = phd.tile([128, D], F32, tag="o4")
                    nc.vector.tensor_mul(out=o4, in0=o3, in1=lg_sb)
                    o5 = phd.tile([128, D], F32, tag="o5")
                    nc.vector.tensor_add(out=o5, in0=o4, in1=lb_sb)
                    nc.sync.dma_start(out=out_d[b], in_=o5)

    nc.compile()
    return nc


def _build():
    """Fallback build for arbitrary (non-Toeplitz) pos_emb: original
    sequence-sharded kernel with full per-query r-projection."""
    nc = bacc.Bacc()

    # ---- DRAM I/O (per-core shapes; data differs per core) ----
    pe_d = nc.dram_tensor("pe", [Q, D, S], F16, kind="ExternalInput")      # pos_emb slice, D-major
    kt_d = nc.dram_tensor("kt", [B, D, S], F16, kind="ExternalInput")      # k transposed
    vt_d = nc.dram_tensor("vt", [B, D, S], F16, kind="ExternalInput")      # v transposed
    qt_d = nc.dram_tensor("qt", [128, NCH, B, Q], F16, kind="ExternalInput")  # q slice, SBUF layout
    qn_d = nc.dram_tensor("qn", [B, Q, D], F32, kind="ExternalInput")      # q slice natural (residual)
    wq_d = nc.dram_tensor("wqt", [D, D], F16, kind="ExternalInput")        # Wq.T  [Din, dout]
    wk_d = nc.dram_tensor("wkt", [D, D], F16, kind="ExternalInput")
    wv_d = nc.dram_tensor("wvt", [D, D], F16, kind="ExternalInput")
    wr_d = nc.dram_tensor("wrt", [D, D], F16, kind="ExternalInput")
    wo_d = nc.dram_tensor("wot", [D, D], F16, kind="ExternalInput")
    bu_d = nc.dram_tensor("bu", [128, NCH], F32, kind="ExternalInput")     # bq+u_bias, [Dinner, chunk]
    bv_d = nc.dram_tensor("bv2", [128, NCH], F32, kind="ExternalInput")    # bq+v_bias
    bo_d = nc.dram_tensor("bo2", [1, D], F32, kind="ExternalInput")        # bo + Wo@bv
    lg_d = nc.dram_tensor("lng", [1, D], F32, kind="ExternalInput")
    lb_d = nc.dram_tensor("lnb", [1, D], F32, kind="ExternalInput")
    out_d = nc.dram_tensor("out", [B, Q, D], F32, kind="ExternalOutput")
    acb_d = nc.dram_tensor("acb", [Q, B * H, S], F16)                      # ac bounce (re-layout)

    with tile.TileContext(nc) as tc:
        with tc.tile_pool(name="consts", bufs=1) as consts:
            ident = consts.tile([128, 128], F16)
            make_identity(nc, ident)

            wr_sb = consts.tile([128, NCH, D], F16, tag="wr")
            nc.sync.dma_start(out=wr_sb, in_=wr_d[:].rearrange("(c p) j -> p c j", p=128))
            wo_sb = consts.tile([128, NCH, D], F16, tag="wo")
            nc.sync.dma_start(out=wo_sb, in_=wo_d[:].rearrange("(c p) j -> p c j", p=128))
            wv_sb = consts.tile([128, NCH, D], F16, tag="wv")
            nc.sync.dma_start(out=wv_sb, in_=wv_d[:].rearrange("(c p) j -> p c j", p=128))

            bu_ld = consts.tile([128, NCH], F32, tag="buld")
            nc.sync.dma_start(out=bu_ld, in_=bu_d[:])
            bv_ld = consts.tile([128, NCH], F32, tag="bvld")
            nc.sync.dma_start(out=bv_ld, in_=bv_d[:])
            # DVE-local copies: keeps scalar-AP consumers to a single sync wait
            bu_sb = consts.tile([128, NCH], F32, tag="bu")
            nc.vector.tensor_copy(out=bu_sb, in_=bu_ld)
            bv_sb = consts.tile([128, NCH], F32, tag="bv")
            nc.vector.tensor_copy(out=bv_sb, in_=bv_ld)
            bo_sb = consts.tile([Q, D], F32, tag="bo")
            nc.sync.dma_start(out=bo_sb, in_=bo_d[:].to_broadcast((Q, D)))
            lg_sb = consts.tile([Q, D], F32, tag="lg")
            nc.sync.dma_start(out=lg_sb, in_=lg_d[:].to_broadcast((Q, D)))
            lb_sb = consts.tile([Q, D], F32, tag="lb")
            nc.sync.dma_start(out=lb_sb, in_=lb_d[:].to_broadcast((Q, D)))
            eps_sb = consts.tile([Q, 1], F32, tag="eps")
            nc.vector.memset(eps_sb, LN_EPS)

            qv2 = consts.tile([128, NCH, Q, 64], F16, tag="qv2")    # block-diag lhsT for bd
            attn_all = consts.tile([128, NCH, 64, Q], F16, tag="attn")  # [k_in, kc, bh, q]

            # ---------------- Phase B: projections + ac ----------------
            with tc.tile_pool(name="phb", bufs=2) as phb, \
                 tc.tile_pool(name="phb1", bufs=1) as phb1, \
                 tc.tile_pool(name="psb", bufs=2, space="PSUM") as psb:
                wq_sb = phb1.tile([128, NCH, D], F16, tag="wqt")
                nc.sync.dma_start(out=wq_sb, in_=wq_d[:].rearrange("(c p) j -> p c j", p=128))
                wk_sb = phb1.tile([128, NCH, D], F16, tag="wkt")
                nc.sync.dma_start(out=wk_sb, in_=wk_d[:].rearrange("(c p) j -> p c j", p=128))
                qt_sb = phb1.tile([128, NCH, B, Q], F16, tag="qt")
                nc.sync.dma_start(out=qt_sb, in_=qt_d[:])
                qu_all = phb1.tile([128, NCH, B, Q], F16, tag="qu")
                qv_all = phb1.tile([128, NCH, B, Q], F16, tag="qv")

                for b in range(B):
                    # qh projection -> qu/qv (+biases), fp16
                    for m in range(NCH):
                        ps_q = psb.tile([128, Q], F32, tag="psq")
                        for c in range(NCH):
                            nc.tensor.matmul(ps_q, wq_sb[:, c, m * 128:(m + 1) * 128],
                                             qt_sb[:, c, b, :], start=(c == 0), stop=(c == NCH - 1))
                        nc.scalar.activation(out=qu_all[:, m, b, :], in_=ps_q,
                                             func=mybir.ActivationFunctionType.Identity,
                                             bias=bu_sb[:, m:m + 1])
                        nc.scalar.activation(out=qv_all[:, m, b, :], in_=ps_q,
                                             func=mybir.ActivationFunctionType.Identity,
                                             bias=bv_sb[:, m:m + 1])

                    kt_sb = phb.tile([128, NCH, S], F16, tag="ktl")
                    nc.sync.dma_start(out=kt_sb, in_=kt_d[b].rearrange("(c p) j -> p c j", p=128))
                    kh_sb = phb.tile([128, NCH, S], F16, tag="kh")
                    for m in range(NCH):
                        ps_k = psb.tile([128, S], F32, tag="psk")
                        for c in range(NCH):
                            nc.tensor.matmul(ps_k, wk_sb[:, c, m * 128:(m + 1) * 128],
                                             kt_sb[:, c, :], start=(c == 0), stop=(c == NCH - 1))
                        nc.vector.tensor_copy(out=kh_sb[:, m, :], in_=ps_k)
                    # ac scores, one matmul per head: [q, k] -> DRAM bounce [q, bh, k]
                    for h in range(H):
                        ps_ac = psb.tile([Q, S], F32, tag="psac")
                        po = (h % 2) * 64
                        nc.tensor.matmul(ps_ac,
                                         qu_all[po:po + 64, h // 2, b, :],
                                         kh_sb[po:po + 64, h // 2, :], start=True, stop=True)
                        ac_st = phb.tile([Q, S], F16, tag="acst")
                        if h % 2 == 0:
                            nc.vector.tensor_copy(out=ac_st, in_=ps_ac)
                        else:
                            nc.scalar.copy(out=ac_st, in_=ps_ac)
                        nc.sync.dma_start(out=acb_d[:, h * 8 + b, :], in_=ac_st)

                # build block-diagonal qv2 lhsT: col j = h*8+b, rows = head band
                nc.vector.memset(qv2, 0.0)
                for c in range(NCH):
                    for hh in range(2):
                        h = 2 * c + hh
                        for b in range(B):
                            nc.vector.tensor_copy(
                                out=qv2[hh * 64:hh * 64 + 64, c, :, h * 8 + b],
                                in_=qv_all[hh * 64:hh * 64 + 64, c, b, :])

            # ---------------- Pass 1: per-query r / bd / softmax ----------------
            with tc.tile_pool(name="p1", bufs=3) as p1, \
                 tc.tile_pool(name="p1b", bufs=2) as p1b, \
                 tc.tile_pool(name="ps1", bufs=2, space="PSUM") as ps1:
                for q in range(Q):
                    pet = p1.tile([128, NCH, S], F16, tag="pet")
                    nc.sync.dma_start(out=pet, in_=pe_d[q].rearrange("(c p) j -> p c j", p=128))
                    ac_q = p1.tile([64, S], F16, tag="acq")
                    nc.sync.dma_start(out=ac_q, in_=acb_d[q])
                    r16 = p1b.tile([128, NCH, S], F16, tag="r16")
                    for m in range(NCH):
                        ps_r = ps1.tile([128, S], F32, tag="psr")
                        for c in range(NCH):
                            nc.tensor.matmul(ps_r, wr_sb[:, c, m * 128:(m + 1) * 128],
                                             pet[:, c, :], start=(c == 0), stop=(c == NCH - 1))
                        if m % 2 == 0:
                            nc.vector.tensor_copy(out=r16[:, m, :], in_=ps_r)
                        else:
                            nc.scalar.copy(out=r16[:, m, :], in_=ps_r)
                    # bd scores + ac add (extra identity matmul) -> psum [64, 512]
                    ps_bd = ps1.tile([64, S], F32, tag="psbd")
                    for c in range(NCH):
                        nc.tensor.matmul(ps_bd, qv2[:, c, q, :], r16[:, c, :],
                                         start=(c == 0), stop=False)
                    nc.tensor.matmul(ps_bd, ident[:64, :64], ac_q,
                                     start=False, stop=True)
                    # softmax over k (free axis); scale 1/sqrt(DH)=0.125
                    mx = p1b.tile([64, 1], F32, tag="mx")
                    nc.vector.tensor_reduce(out=mx, in_=ps_bd, axis=mybir.AxisListType.X,
                                            op=mybir.AluOpType.max)
                    nm8 = p1b.tile([64, 1], F32, tag="nm8")
                    nc.vector.tensor_scalar_mul(out=nm8, in0=mx, scalar1=-0.125)
                    pexp = p1b.tile([64, S], F32, tag="pexp")
                    rsum = p1b.tile([64, 1], F32, tag="rsum")
                    nc.scalar.activation(out=pexp, in_=ps_bd,
                                         func=mybir.ActivationFunctionType.Exp,
                                         bias=nm8, scale=0.125, accum_out=rsum)
                    rc = p1b.tile([64, 1], F32, tag="rc")
                    nc.vector.reciprocal(out=rc, in_=rsum)
                    p16 = p1b.tile([64, S], F16, tag="p16")
                    nc.scalar.mul(out=p16, in_=pexp, mul=rc)
                    # transpose attn row-block to [k, bh] and stash
                    ps_at = ps1.tile([128, NCH, 64], F16, tag="psat")
                    for c in range(NCH):
                        nc.tensor.transpose(out=ps_at[:, c, :], in_=p16[:, c * 128:(c + 1) * 128],
                                            identity=ident[:64, :64])
                    nc.vector.tensor_copy(out=attn_all[:, :, :, q], in_=ps_at)

            # ---------------- Pass 2: vh / PV / out-proj / LayerNorm ----------------
            with tc.tile_pool(name="p2", bufs=2) as p2, \
                 tc.tile_pool(name="ps2", bufs=2, space="PSUM") as ps2:
                for b in range(B):
                    vt_sb = p2.tile([128, NCH, S], F16, tag="vtl")
                    nc.sync.dma_start(out=vt_sb, in_=vt_d[b].rearrange("(c p) j -> p c j", p=128))
                    vh_sb = p2.tile([128, NCH, D], F16, tag="vh")  # [k_in, kc, hd]
                    for kc in range(NCH):
                        ps_v = ps2.tile([128, D], F32, tag="psv")
                        for c in range(NCH):
                            nc.tensor.matmul(ps_v, vt_sb[:, c, kc * 128:(kc + 1) * 128],
                                             wv_sb[:, c, :], start=(c == 0), stop=(c == NCH - 1))
                        if kc % 2 == 0:
                            nc.vector.tensor_copy(out=vh_sb[:, kc, :], in_=ps_v)
                        else:
                            nc.scalar.copy(out=vh_sb[:, kc, :], in_=ps_v)
                    aot = p2.tile([128, NCH, Q], F16, tag="aot")   # attn_out.T [hd, q]
                    for h in range(H):
                        ps_ao = ps2.tile([64, Q], F32, tag="psao")
                        for c in range(NCH):
                            nc.tensor.matmul(ps_ao, vh_sb[:, c, h * 64:(h + 1) * 64],
                                             attn_all[:, c, h * 8 + b, :],
                                             start=(c == 0), stop=(c == NCH - 1))
                        po = (h % 2) * 64
                        nc.vector.tensor_copy(out=aot[po:po + 64, h // 2, :], in_=ps_ao)
                    ps_o = ps2.tile([Q, D], F32, tag="pso")
                    for c in range(NCH):
                        nc.tensor.matmul(ps_o, aot[:, c, :], wo_sb[:, c, :],
                                         start=(c == 0), stop=(c == NCH - 1))
                    # residual + bo2 + LayerNorm
                    qn_b = p2.tile([Q, D], F32, tag="qnb")
                    nc.sync.dma_start(out=qn_b, in_=qn_d[b])
                    o1 = p2.tile([Q, D], F32, tag="o1")
                    nc.vector.tensor_add(out=o1, in0=ps_o, in1=qn_b)
                    o2 = p2.tile([Q, D], F32, tag="o2")
                    nc.vector.tensor_add(out=o2, in0=o1, in1=bo_sb)
                    st6 = p2.tile([Q, nc.vector.BN_STATS_DIM], F32, tag="st6")
                    nc.vector.bn_stats(out=st6, in_=o2)
                    mv = p2.tile([Q, nc.vector.BN_AGGR_DIM], F32, tag="mv")
                    nc.vector.bn_aggr(out=mv, in_=st6)
                    sd = p2.tile([Q, 1], F32, tag="sd")
                    nc.scalar.activation(out=sd, in_=mv[:, 1:2],
                                         func=mybir.ActivationFunctionType.Sqrt,
                                         bias=eps_sb, scale=1.0)
                    rstd = p2.tile([Q, 1], F32, tag="rstd")
                    nc.vector.reciprocal(out=rstd, in_=sd)
                    mr = p2.tile([Q, 1], F32, tag="mr")
                    nc.vector.tensor_mul(out=mr, in0=mv[:, 0:1], in1=rstd)
                    nmr = p2.tile([Q, 1], F32, tag="nmr")
                    nc.vector.tensor_scalar_mul(out=nmr, in0=mr, scalar1=-1.0)
                    o3 = p2.tile([Q, D], F32, tag="o3")
                    nc.scalar.activation(out=o3, in_=o2,
                                         func=mybir.ActivationFunctionType.Identity,
                                         bias=nmr, scale=rstd)
                    o4 = p2.tile([Q, D], F32, tag="o4")
                    nc.vector.tensor_mul(out=o4, in0=o3, in1=lg_sb)
                    o5 = p2.tile([Q, D], F32, tag="o5")
                    nc.vector.tensor_add(out=o5, in0=o4, in1=lb_sb)
                    nc.sync.dma_start(out=out_d[b], in_=o5)

    nc.compile()
    return nc


def _toeplitz_table(pos):
    """Extract the 1023-row relative-position table if pos_emb has the
    Transformer-XL structure pos[q, k] == table[k - q + 511]; else None."""
    if pos.shape != (S, S, D):
        return None
    table = np.concatenate([pos[S - 1:0:-1, 0, :], pos[0, :, :]], axis=0)
    rng = np.random.default_rng(12345)
    qs = rng.integers(0, S, 96)
    ks = rng.integers(0, S, 96)
    for q, k in zip(qs, ks):
        if not np.array_equal(pos[q, k], table[k - q + S - 1]):
            return None
    return table


def kernel(**inputs):
    global last_result
    f16, f32 = np.float16, np.float32
    q = np.asarray(inputs["q"], f32)
    k = np.asarray(inputs["k"], f32)
    v = np.asarray(inputs["v"], f32)
    pos = np.asarray(inputs["pos_emb"], f32)
    Wq, Wk, Wv, Wr, Wo = (np.asarray(inputs[n], f32) for n in ("Wq", "Wk", "Wv", "Wr", "Wo"))
    bq, bo, bvb = (np.asarray(inputs[n], f32) for n in ("bq", "bo", "bv"))
    u_b = np.asarray(inputs["u_bias"], f32).reshape(-1)
    v_b = np.asarray(inputs["v_bias"], f32).reshape(-1)
    lng, lnb = np.asarray(inputs["ln_g"], f32), np.asarray(inputs["ln_b"], f32)

    bo2 = (bo + Wo @ bvb).astype(f32)

    table = _toeplitz_table(pos)
    if table is not None:
        if "nc2d" not in _CACHE:
            _CACHE["nc2d"] = _build_2d()
        nc = _CACHE["nc2d"]

        # shared staging
        tbl_pad = np.zeros((JW, D), f32)
        tbl_pad[:2 * S - 1] = table
        pet = np.ascontiguousarray(
            tbl_pad.T.reshape(NCH, 128, JW).transpose(1, 0, 2)).astype(f16)
        bu_full = (bq + u_b)
        bv_full = (bq + v_b)
        wqt, wkt, wvt, wrt = (np.ascontiguousarray(W.T) for W in (Wq, Wk, Wv, Wr))
        wot = np.ascontiguousarray(Wo.T)
        lg_s = lng.reshape(1, D).astype(f32)
        lb_s = lnb.reshape(1, D).astype(f32)

        def dmajor(x):  # [BL, S, D] -> [128, NCH, BL, S]
            return np.ascontiguousarray(
                x.transpose(0, 2, 1).reshape(BL, NCH, 128, S).transpose(2, 1, 0, 3)
            ).astype(f16)

        def wpair(wt, hd):  # W.T[:, hd] -> [128, NCH, 128]
            return np.ascontiguousarray(
                wt[:, hd].reshape(NCH, 128, 128).transpose(1, 0, 2)).astype(f16)

        in_maps = []
        for c in range(NCORES):
            bg, hg = c // 4, c % 4
            bs = slice(bg * BL, (bg + 1) * BL)
            hd = slice(hg * 128, (hg + 1) * 128)
            in_maps.append(dict(
                qt=dmajor(q[bs]), kt=dmajor(k[bs]), vt=dmajor(v[bs]),
                pet=pet,
                wqp=wpair(wqt, hd), wkp=wpair(wkt, hd),
                wvp=wpair(wvt, hd), wrp=wpair(wrt, hd),
                wop=np.ascontiguousarray(wot[hd, :]).astype(f16),
                bu=np.ascontiguousarray(bu_full[hd]).reshape(128, 1).astype(f32),
                bv2=np.ascontiguousarray(bv_full[hd]).reshape(128, 1).astype(f32),
                qn=np.ascontiguousarray(q[bs, hd, :] + bo2).astype(f32),
                lng=lg_s, lnb=lb_s,
            ))
        res = run_bass_kernel_spmd(nc, in_maps, core_ids=list(range(NCORES)))
        last_result = res
        out = np.empty((B, S, D), f32)
        for c in range(NCORES):
            bg, hg = c // 4, c % 4
            out[bg * BL:(bg + 1) * BL, hg * 128:(hg + 1) * 128, :] = res.results[c]["out"]
        return out

    # ---------------- fallback: arbitrary pos_emb ----------------
    if "nc" not in _CACHE:
        _CACHE["nc"] = _build()
    nc = _CACHE["nc"]
    wqt = np.ascontiguousarray(Wq.T).astype(f16)
    wkt = np.ascontiguousarray(Wk.T).astype(f16)
    wvt = np.ascontiguousarray(Wv.T).astype(f16)
    wrt = np.ascontiguousarray(Wr.T).astype(f16)
    wot = np.ascontiguousarray(Wo.T).astype(f16)
    bu = np.ascontiguousarray((bq + u_b).reshape(NCH, 128).T).astype(f32)
    bv2 = np.ascontiguousarray((bq + v_b).reshape(NCH, 128).T).astype(f32)
    kt = np.ascontiguousarray(k.transpose(0, 2, 1)).astype(f16)
    vt = np.ascontiguousarray(v.transpose(0, 2, 1)).astype(f16)
    qt_full = np.ascontiguousarray(q.transpose(0, 2, 1)).astype(f16)   # [B, D, S]
    shared = dict(kt=kt, vt=vt, wqt=wqt, wkt=wkt, wvt=wvt, wrt=wrt, wot=wot,
                  bu=bu, bv2=bv2, bo2=bo2.reshape(1, D),
                  lng=lng.reshape(1, D).astype(f32), lnb=lnb.reshape(1, D).astype(f32))
    in_maps = []
    pos_t = pos.transpose(0, 2, 1)                                 # view [q, D, k]
    for c in range(NCORES):
        sl = slice(c * Q, (c + 1) * Q)
        qt_c = qt_full[:, :, sl].reshape(B, NCH, 128, Q).transpose(2, 1, 0, 3)
        in_maps.append(dict(shared,
                            pe=np.ascontiguousarray(pos_t[sl]).astype(f16),
                            qt=np.ascontiguousarray(qt_c),
                            qn=np.ascontiguousarray(q[:, sl, :])))

    res = run_bass_kernel_spmd(nc, in_maps, core_ids=list(range(NCORES)))
    last_result = res
    out = np.concatenate([r["out"] for r in res.results], axis=1)
    return out.astype(f32)


# revision 9
# speedup vs baseline: 1.4475x; 1.4475x over previous
"""Trainium2 Bass kernel for HarmonyTransformer (relative-position attention block).

Sharding (fast path): 2-D mesh over 8 NeuronCores = 4 head-groups x 2
batch-groups. Core c handles batches [bg*4, bg*4+4) (bg = c//4) and heads
{2*hg, 2*hg+1} (hg = c%4) for the FULL sequence. q/k/v/r projections are
sliced 8-fold (vs replicated in a pure sequence shard); the only cross-core
step is a bf16 ReduceScatter (groups of 4 cores sharing a batch group) that
sums the per-head-group output-projection partials and hands each core one
sequence quarter, on which it applies residual + LayerNorm locally.

Fast path requires the Transformer-XL Toeplitz structure
pos_emb[q,k] = table[k-q+511] (reference.setup_inputs always produces it):
r = table @ Wr_pair.T is computed once per core ([128 dh, 1023]); bd scores
are computed in table space per 128-query chunk ([128, 640] windows) and
re-read per-query-shifted through a DRAM bounce using a pitch-639 strided
view (row r starts 1 element earlier than row r-1).

Math identities used (exact):
  - bk and br shift every score in a softmax row by a constant -> dropped.
  - bv passes through attention and Wo linearly: bo2 = bo + Wo @ bv, and
    bo2 is pre-added to the residual on the host.
  - bq folded into per-head biases: biasU = bq + u_bias, biasV = bq + v_bias.
  - softmax normalization deferred: unnormalized exp scores are transposed
    on the PE with rhs = diag(1/rowsum) instead of the identity, making the
    normalization free.

Device compute is fp16 multiplies with fp32 PSUM accumulation. A structure
spot-check on the host picks the fast path; arbitrary pos_emb falls back to
the original sequence-sharded kernel (_build).
"""

import numpy as np

import concourse.bass as bass
import concourse.bacc as bacc
import concourse.mybir as mybir
import concourse.tile as tile
from concourse.masks import make_identity
from concourse.bass_utils import run_bass_kernel_spmd

B, S, D, H, DH = 8, 512, 512, 8, 64
NCORES = 8
Q = S // NCORES            # fallback kernel: 64 queries per core
NCH = 4                    # 128-partition chunks of D
F32 = mybir.dt.float32
F16 = mybir.dt.float16
BF16 = mybir.dt.bfloat16
LN_EPS = 1e-5
TW = 575                   # fallback: per-core table-slice rows
TWP = 576
BDP = 640

# fast path (2-D shard) constants
BL = 4                     # local batches per core
JW = 1024                  # padded table width (1023 rows + 1 pad)
PITCH = 640                # bd_raw chunk window width / DRAM row pitch

_CACHE = {}
last_result = None


def _build_2d():
    nc = bacc.Bacc()

    # ---- DRAM I/O (per-core shapes; data differs per core) ----
    qt_d = nc.dram_tensor("qt", [128, NCH, BL, S], F16, kind="ExternalInput")
    kt_d = nc.dram_tensor("kt", [128, NCH, BL, S], F16, kind="ExternalInput")
    vt_d = nc.dram_tensor("vt", [128, NCH, BL, S], F16, kind="ExternalInput")
    pet_d = nc.dram_tensor("pet", [128, NCH, JW], F16, kind="ExternalInput")
    wq_d = nc.dram_tensor("wqp", [128, NCH, 128], F16, kind="ExternalInput")
    wk_d = nc.dram_tensor("wkp", [128, NCH, 128], F16, kind="ExternalInput")
    wv_d = nc.dram_tensor("wvp", [128, NCH, 128], F16, kind="ExternalInput")
    wr_d = nc.dram_tensor("wrp", [128, NCH, 128], F16, kind="ExternalInput")
    wo_d = nc.dram_tensor("wop", [128, D], F16, kind="ExternalInput")
    bu_d = nc.dram_tensor("bu", [128, 1], F32, kind="ExternalInput")
    bv_d = nc.dram_tensor("bv2", [128, 1], F32, kind="ExternalInput")
    qn_d = nc.dram_tensor("qn", [BL, 128, D], F32, kind="ExternalInput")
    lg_d = nc.dram_tensor("lng", [1, D], F32, kind="ExternalInput")
    lb_d = nc.dram_tensor("lnb", [1, D], F32, kind="ExternalInput")
    out_d = nc.dram_tensor("out", [BL, 128, D], F32, kind="ExternalOutput")
    bdr_bs = [nc.dram_tensor(f"bdr{b}", [2 * NCH * 128 * PITCH], F16)
              for b in range(BL)]

    with tile.TileContext(nc) as tc:
        with tc.tile_pool(name="consts", bufs=1) as consts, \
             tc.tile_pool(name="dpool", bufs=1, space="DRAM") as dpool:
            rs_ins = [dpool.tile([NCH, 128, D], BF16, tag=f"rsin{b}",
                                 name=f"rsin{b}") for b in range(BL)]
            rs_outs = [dpool.tile([128, D], BF16, tag=f"rsout{b}",
                                  name=f"rsout{b}") for b in range(BL)]

            ident = consts.tile([128, 128], F16)
            make_identity(nc, ident)

            wo_sb = consts.tile([128, D], F16, tag="wo")
            nc.sync.dma_start(out=wo_sb, in_=wo_d[:])
            bu_ld = consts.tile([128, 1], F32, tag="buld")
            nc.sync.dma_start(out=bu_ld, in_=bu_d[:])
            bv_ld = consts.tile([128, 1], F32, tag="bvld")
            nc.sync.dma_start(out=bv_ld, in_=bv_d[:])
            bu_sb = consts.tile([128, 1], F32, tag="bu")
            nc.vector.tensor_copy(out=bu_sb, in_=bu_ld)
            bv_sb = consts.tile([128, 1], F32, tag="bv")
            nc.vector.tensor_copy(out=bv_sb, in_=bv_ld)
            lg_sb = consts.tile([128, D], F32, tag="lg")
            nc.sync.dma_start(out=lg_sb, in_=lg_d[:].to_broadcast((128, D)))
            lb_sb = consts.tile([128, D], F32, tag="lb")
            nc.sync.dma_start(out=lb_sb, in_=lb_d[:].to_broadcast((128, D)))
            eps_sb = consts.tile([128, 1], F32, tag="eps")
            nc.vector.memset(eps_sb, LN_EPS)

            rt_sb = consts.tile([128, JW], F16, tag="rt")        # [dh-pair, j]
            qu_sb = consts.tile([128, BL, S], F16, tag="qu")     # [dh-pair, b, q]
            qv_sb = consts.tile([128, BL, S], F16, tag="qv")
            kh_sb = consts.tile([128, BL, S], F16, tag="kh")     # [dh-pair, b, k]
            vh_sb = consts.tile([128, BL, NCH, 128], F16, tag="vh")  # [k_in, b, kc, dh]

            # ---------------- Phase A: projections ----------------
            with tc.tile_pool(name="pha", bufs=1) as pha, \
                 tc.tile_pool(name="psA", bufs=3, space="PSUM") as psA:
                wr_sb = pha.tile([128, NCH, 128], F16, tag="wr")
                nc.sync.dma_start(out=wr_sb, in_=wr_d[:])
                pet_sb = pha.tile([128, NCH, JW], F16, tag="pet")
                nc.sync.dma_start(out=pet_sb, in_=pet_d[:])
                wq_sb = pha.tile([128, NCH, 128], F16, tag="wq")
                nc.sync.dma_start(out=wq_sb, in_=wq_d[:])
                wk_sb = pha.tile([128, NCH, 128], F16, tag="wk")
                nc.sync.dma_start(out=wk_sb, in_=wk_d[:])
                wv_sb = pha.tile([128, NCH, 128], F16, tag="wv")
                nc.sync.dma_start(out=wv_sb, in_=wv_d[:])
                qt_sb = pha.tile([128, NCH, BL, S], F16, tag="qt")
                nc.sync.dma_start(out=qt_sb, in_=qt_d[:])
                kt_sb = pha.tile([128, NCH, BL, S], F16, tag="kt")
                nc.sync.dma_start(out=kt_sb, in_=kt_d[:])
                vt_sb = pha.tile([128, NCH, BL, S], F16, tag="vt")
                nc.sync.dma_start(out=vt_sb, in_=vt_d[:])

                # r-table projection: rt = Wr_pair.T-proj of table  [128 dh, JW]
                for jh in range(2):
                    ps_r = psA.tile([128, 512], F32, tag="psA")
                    for c in range(NCH):
                        nc.tensor.matmul(ps_r, wr_sb[:, c, :],
                                         pet_sb[:, c, jh * 512:(jh + 1) * 512],
                                         start=(c == 0), stop=(c == NCH - 1))
                    if jh == 0:
                        nc.vector.tensor_copy(out=rt_sb[:, 0:512], in_=ps_r)
                    else:
                        nc.scalar.copy(out=rt_sb[:, 512:JW], in_=ps_r)

                # q projection (+ biases) -> qu / qv
                for b in range(BL):
                    ps_q = psA.tile([128, 512], F32, tag="psA")
                    for c in range(NCH):
                        nc.tensor.matmul(ps_q, wq_sb[:, c, :], qt_sb[:, c, b, :],
                                         start=(c == 0), stop=(c == NCH - 1))
                    nc.scalar.activation(out=qu_sb[:, b, :], in_=ps_q,
                                         func=mybir.ActivationFunctionType.Identity,
                                         bias=bu_sb)
                    nc.scalar.activation(out=qv_sb[:, b, :], in_=ps_q,
                                         func=mybir.ActivationFunctionType.Identity,
                                         bias=bv_sb)

                # k projection
                for b in range(BL):
                    ps_k = psA.tile([128, 512], F32, tag="psA")
                    for c in range(NCH):
                        nc.tensor.matmul(ps_k, wk_sb[:, c, :], kt_sb[:, c, b, :],
                                         start=(c == 0), stop=(c == NCH - 1))
                    nc.vector.tensor_copy(out=kh_sb[:, b, :], in_=ps_k)

                # v projection in [dh, k] layout, then DMA-transpose to
                # vh [k_in, b, kc, dh-pair]
                for b in range(BL):
                    ps_v = psA.tile([128, 512], F32, tag="psA")
                    for c in range(NCH):
                        nc.tensor.matmul(ps_v, wv_sb[:, c, :], vt_sb[:, c, b, :],
                                         start=(c == 0), stop=(c == NCH - 1))
                    vht = pha.tile([128, 512], F16, tag=f"vht{b % 2}")
                    if b % 2 == 0:
                        nc.vector.tensor_copy(out=vht, in_=ps_v)
                    else:
                        nc.scalar.copy(out=vht, in_=ps_v)
                    nc.scalar.dma_start_transpose(out=vh_sb[:, b], in_=vht[:])

            # ------- Phases B-D fused per batch: bd_raw -> scores/softmax ->
            # ------- PV -> out-proj -> chunked ReduceScatter ----------------
            with tc.tile_pool(name="phb", bufs=3) as phb, \
                 tc.tile_pool(name="phc", bufs=4) as phc, \
                 tc.tile_pool(name="phc2", bufs=3) as phc2, \
                 tc.tile_pool(name="phd", bufs=3) as phd, \
                 tc.tile_pool(name="attnp", bufs=2) as attnp, \
                 tc.tile_pool(name="ps512", bufs=3, space="PSUM") as ps512, \
                 tc.tile_pool(name="ps128", bufs=1, space="PSUM") as ps128, \
                 tc.tile_pool(name="psT", bufs=2, space="PSUM") as psT, \
                 tc.tile_pool(name="psD", bufs=1, space="PSUM") as psD, \
                 tc.tile_pool(name="psO", bufs=1, space="PSUM") as psO:
                for b in range(BL):
                    bdr_w = bdr_bs[b][:].rearrange("(h c q j) -> h c q j",
                                                   h=2, c=NCH, q=128, j=PITCH)
                    # bd_raw chunks -> DRAM bounce
                    i = 0
                    for qc in range(NCH):
                        jc = 384 - 128 * qc
                        for h in range(2):
                            lhs = qv_sb[h * 64:(h + 1) * 64, b, qc * 128:(qc + 1) * 128]
                            ps1 = ps512.tile([128, 512], F32, tag="ps512")
                            nc.tensor.matmul(ps1, lhs,
                                             rt_sb[h * 64:(h + 1) * 64, jc:jc + 512],
                                             start=True, stop=True)
                            ps2 = ps128.tile([128, 128], F32, tag="ps128")
                            nc.tensor.matmul(ps2, lhs,
                                             rt_sb[h * 64:(h + 1) * 64, jc + 512:jc + PITCH],
                                             start=True, stop=True)
                            bd_st = phb.tile([128, PITCH], F16, tag="bdst")
                            if i % 2 == 0:
                                nc.vector.tensor_copy(out=bd_st[:, 0:512], in_=ps1)
                                nc.scalar.copy(out=bd_st[:, 512:PITCH], in_=ps2)
                            else:
                                nc.scalar.copy(out=bd_st[:, 0:512], in_=ps1)
                                nc.vector.tensor_copy(out=bd_st[:, 512:PITCH], in_=ps2)
                            nc.sync.dma_start(out=bdr_w[h, qc], in_=bd_st)
                            i += 1

                    # scores + softmax + transposed attn
                    attn = attnp.tile([128, NCH, 2, S], F16, tag="attn")  # [k_in, kc, h, q]
                    i = 0
                    for qc in range(NCH):
                        for h in range(2):
                            base = ((h * NCH + qc) * 128) * PITCH + 127
                            bd_sh = phc.tile([128, 512], F16, tag="bdsh")
                            nc.scalar.dma_start(
                                out=bd_sh,
                                in_=bdr_bs[b][base:base + 128 * (PITCH - 1)]
                                .rearrange("(q j) -> q j", j=PITCH - 1)[:, 0:512])
                            ps_sc = ps512.tile([128, 512], F32, tag="ps512")
                            nc.tensor.matmul(ps_sc,
                                             qu_sb[h * 64:(h + 1) * 64, b, qc * 128:(qc + 1) * 128],
                                             kh_sb[h * 64:(h + 1) * 64, b, :],
                                             start=True, stop=False)
                            nc.tensor.matmul(ps_sc, ident, bd_sh,
                                             start=False, stop=True)
                            # softmax over k (free axis); scale 1/sqrt(DH)=0.125.
                            # No max subtraction: |score|/8 is far inside exp range.
                            p16 = phc2.tile([128, 512], F16, tag="p16")
                            rsum = phc2.tile([128, 1], F32, tag="rsum")
                            nc.scalar.activation(out=p16, in_=ps_sc,
                                                 func=mybir.ActivationFunctionType.Exp,
                                                 scale=0.125, accum_out=rsum)
                            rc = phc2.tile([128, 1], F32, tag="rc")
                            nc.vector.reciprocal(out=rc, in_=rsum)
                            dg = phc2.tile([128, 128], F16, tag="dg")
                            nc.vector.tensor_scalar_mul(out=dg, in0=ident, scalar1=rc)
                            # transpose q-rows -> k-partitions, normalizing via diag
                            ps_at = psT.tile([128, NCH, 128], F32, tag="psat")
                            for kc in range(NCH):
                                nc.tensor.matmul(ps_at[:, kc, :],
                                                 p16[:, kc * 128:(kc + 1) * 128], dg,
                                                 start=True, stop=True)
                            if i % 2 == 0:
                                nc.vector.tensor_copy(
                                    out=attn[:, :, h, qc * 128:(qc + 1) * 128],
                                    in_=ps_at)
                            else:
                                nc.scalar.copy(
                                    out=attn[:, :, h, qc * 128:(qc + 1) * 128],
                                    in_=ps_at)
                            i += 1

                    # PV: [dh-pair, q] accumulated over k chunks
                    ps_pv = psD.tile([128, 512], F32, tag="pspv")
                    for h in range(2):
                        for kc in range(NCH):
                            nc.tensor.matmul(ps_pv[h * 64:(h + 1) * 64, :],
                                             vh_sb[:, b, kc, h * 64:(h + 1) * 64],
                                             attn[:, kc, h, :],
                                             start=(kc == 0), stop=(kc == NCH - 1))
                    aot = phd.tile([128, 512], F16, tag="aot")
                    if b % 2 == 0:
                        nc.vector.tensor_copy(out=aot, in_=ps_pv)
                    else:
                        nc.scalar.copy(out=aot, in_=ps_pv)
                    # out-projection partials -> bf16 -> per-batch ReduceScatter
                    for qc in range(NCH):
                        ps_o = psO.tile([128, 512], F32, tag="pso")
                        nc.tensor.matmul(ps_o, aot[:, qc * 128:(qc + 1) * 128], wo_sb,
                                         start=True, stop=True)
                        ro = phd.tile([128, 512], BF16, tag="ro")
                        if (b + qc) % 2 == 0:
                            nc.scalar.copy(out=ro, in_=ps_o)
                        else:
                            nc.vector.tensor_copy(out=ro, in_=ps_o)
                        nc.sync.dma_start(out=rs_ins[b][qc], in_=ro)
                    nc.gpsimd.collective_compute(
                        "ReduceScatter", mybir.AluOpType.add,
                        replica_groups=[[0, 1, 2, 3], [4, 5, 6, 7]],
                        ins=[rs_ins[b].opt()], outs=[rs_outs[b].opt()])

            # ---------------- Epilogue: residual + LayerNorm ----------------
            with tc.tile_pool(name="phe", bufs=3) as phd:
                for b in range(BL):
                    rr = phd.tile([128, D], BF16, tag="rr")
                    nc.sync.dma_start(out=rr, in_=rs_outs[b][:])
                    qn_b = phd.tile([128, D], F32, tag="qnb")
                    nc.sync.dma_start(out=qn_b, in_=qn_d[b])
                    o1 = phd.tile([128, D], F32, tag="o1")
                    nc.vector.tensor_add(out=o1, in0=rr, in1=qn_b)
                    st6 = phd.tile([128, nc.vector.BN_STATS_DIM], F32, tag="st6")
                    nc.vector.bn_stats(out=st6, in_=o1)
                    mv = phd.tile([128, nc.vector.BN_AGGR_DIM], F32, tag="mv")
                    nc.vector.bn_aggr(out=mv, in_=st6)
                    sd = phd.tile([128, 1], F32, tag="sd")
                    nc.scalar.activation(out=sd, in_=mv[:, 1:2],
                                         func=mybir.ActivationFunctionType.Sqrt,
                                         bias=eps_sb, scale=1.0)
                    rstd = phd.tile([128, 1], F32, tag="rstd")
                    nc.vector.reciprocal(out=rstd, in_=sd)
                    mr = phd.tile([128, 1], F32, tag="mr")
                    nc.vector.tensor_mul(out=mr, in0=mv[:, 0:1], in1=rstd)
                    nmr = phd.tile([128, 1], F32, tag="nmr")
                    nc.vector.tensor_scalar_mul(out=nmr, in0=mr, scalar1=-1.0)
                    o3 = phd.tile([128, D], F32, tag="o3")
                    nc.scalar.activation(out=o3, in_=o1,
                                         func=mybir.ActivationFunctionType.Identity,
                                         bias=nmr, scale=rstd)
                    o4 = phd.tile([128, D], F32, tag="o4")
                    nc.vector.tensor_mul(out=o4, in0=o3, in1=lg_sb)
                    o5 = phd.tile([128, D], F32, tag="o5")
                    nc.vector.tensor_add(out=o5, in0=o4, in1=lb_sb)
                    nc.sync.dma_start(out=out_d[b], in_=o5)

    nc.compile()
    return nc


def _build():
    """Fallback build for arbitrary (non-Toeplitz) pos_emb: original
    sequence-sharded kernel with full per-query r-projection."""
    nc = bacc.Bacc()

    # ---- DRAM I/O (per-core shapes; data differs per core) ----
    pe_d = nc.dram_tensor("pe", [Q, D, S], F16, kind="ExternalInput")      # pos_emb slice, D-major
    kt_d = nc.dram_tensor("kt", [B, D, S], F16, kind="ExternalInput")      # k transposed
    vt_d = nc.dram_tensor("vt", [B, D, S], F16, kind="ExternalInput")      # v transposed
    qt_d = nc.dram_tensor("qt", [128, NCH, B, Q], F16, kind="ExternalInput")  # q slice, SBUF layout
    qn_d = nc.dram_tensor("qn", [B, Q, D], F32, kind="ExternalInput")      # q slice natural (residual)
    wq_d = nc.dram_tensor("wqt", [D, D], F16, kind="ExternalInput")        # Wq.T  [Din, dout]
    wk_d = nc.dram_tensor("wkt", [D, D], F16, kind="ExternalInput")
    wv_d = nc.dram_tensor("wvt", [D, D], F16, kind="ExternalInput")
    wr_d = nc.dram_tensor("wrt", [D, D], F16, kind="ExternalInput")
    wo_d = nc.dram_tensor("wot", [D, D], F16, kind="ExternalInput")
    bu_d = nc.dram_tensor("bu", [128, NCH], F32, kind="ExternalInput")     # bq+u_bias, [Dinner, chunk]
    bv_d = nc.dram_tensor("bv2", [128, NCH], F32, kind="ExternalInput")    # bq+v_bias
    bo_d = nc.dram_tensor("bo2", [1, D], F32, kind="ExternalInput")        # bo + Wo@bv
    lg_d = nc.dram_tensor("lng", [1, D], F32, kind="ExternalInput")
    lb_d = nc.dram_tensor("lnb", [1, D], F32, kind="ExternalInput")
    out_d = nc.dram_tensor("out", [B, Q, D], F32, kind="ExternalOutput")
    acb_d = nc.dram_tensor("acb", [Q, B * H, S], F16)                      # ac bounce (re-layout)

    with tile.TileContext(nc) as tc:
        with tc.tile_pool(name="consts", bufs=1) as consts:
            ident = consts.tile([128, 128], F16)
            make_identity(nc, ident)

            wr_sb = consts.tile([128, NCH, D], F16, tag="wr")
            nc.sync.dma_start(out=wr_sb, in_=wr_d[:].rearrange("(c p) j -> p c j", p=128))
            wo_sb = consts.tile([128, NCH, D], F16, tag="wo")
            nc.sync.dma_start(out=wo_sb, in_=wo_d[:].rearrange("(c p) j -> p c j", p=128))
            wv_sb = consts.tile([128, NCH, D], F16, tag="wv")
            nc.sync.dma_start(out=wv_sb, in_=wv_d[:].rearrange("(c p) j -> p c j", p=128))

            bu_ld = consts.tile([128, NCH], F32, tag="buld")
            nc.sync.dma_start(out=bu_ld, in_=bu_d[:])
            bv_ld = consts.tile([128, NCH], F32, tag="bvld")
            nc.sync.dma_start(out=bv_ld, in_=bv_d[:])
            # DVE-local copies: keeps scalar-AP consumers to a single sync wait
            bu_sb = consts.tile([128, NCH], F32, tag="bu")
            nc.vector.tensor_copy(out=bu_sb, in_=bu_ld)
            bv_sb = consts.tile([128, NCH], F32, tag="bv")
            nc.vector.tensor_copy(out=bv_sb, in_=bv_ld)
            bo_sb = consts.tile([Q, D], F32, tag="bo")
            nc.sync.dma_start(out=bo_sb, in_=bo_d[:].to_broadcast((Q, D)))
            lg_sb = consts.tile([Q, D], F32, tag="lg")
            nc.sync.dma_start(out=lg_sb, in_=lg_d[:].to_broadcast((Q, D)))
            lb_sb = consts.tile([Q, D], F32, tag="lb")
            nc.sync.dma_start(out=lb_sb, in_=lb_d[:].to_broadcast((Q, D)))
            eps_sb = consts.tile([Q, 1], F32, tag="eps")
            nc.vector.memset(eps_sb, LN_EPS)

            qv2 = consts.tile([128, NCH, Q, 64], F16, tag="qv2")    # block-diag lhsT for bd
            attn_all = consts.tile([128, NCH, 64, Q], F16, tag="attn")  # [k_in, kc, bh, q]

            # ---------------- Phase B: projections + ac ----------------
            with tc.tile_pool(name="phb", bufs=2) as phb, \
                 tc.tile_pool(name="phb1", bufs=1) as phb1, \
                 tc.tile_pool(name="psb", bufs=2, space="PSUM") as psb:
                wq_sb = phb1.tile([128, NCH, D], F16, tag="wqt")
                nc.sync.dma_start(out=wq_sb, in_=wq_d[:].rearrange("(c p) j -> p c j", p=128))
                wk_sb = phb1.tile([128, NCH, D], F16, tag="wkt")
                nc.sync.dma_start(out=wk_sb, in_=wk_d[:].rearrange("(c p) j -> p c j", p=128))
                qt_sb = phb1.tile([128, NCH, B, Q], F16, tag="qt")
                nc.sync.dma_start(out=qt_sb, in_=qt_d[:])
                qu_all = phb1.tile([128, NCH, B, Q], F16, tag="qu")
                qv_all = phb1.tile([128, NCH, B, Q], F16, tag="qv")

                for b in range(B):
                    # qh projection -> qu/qv (+biases), fp16
                    for m in range(NCH):
                        ps_q = psb.tile([128, Q], F32, tag="psq")
                        for c in range(NCH):
                            nc.tensor.matmul(ps_q, wq_sb[:, c, m * 128:(m + 1) * 128],
                                             qt_sb[:, c, b, :], start=(c == 0), stop=(c == NCH - 1))
                        nc.scalar.activation(out=qu_all[:, m, b, :], in_=ps_q,
                                             func=mybir.ActivationFunctionType.Identity,
                                             bias=bu_sb[:, m:m + 1])
                        nc.scalar.activation(out=qv_all[:, m, b, :], in_=ps_q,
                                             func=mybir.ActivationFunctionType.Identity,
                                             bias=bv_sb[:, m:m + 1])

                    kt_sb = phb.tile([128, NCH, S], F16, tag="ktl")
                    nc.sync.dma_start(out=kt_sb, in_=kt_d[b].rearrange("(c p) j -> p c j", p=128))
                    kh_sb = phb.tile([128, NCH, S], F16, tag="kh")
                    for m in range(NCH):
                        ps_k = psb.tile([128, S], F32, tag="psk")
                        for c in range(NCH):
                            nc.tensor.matmul(ps_k, wk_sb[:, c, m * 128:(m + 1) * 128],
                                             kt_sb[:, c, :], start=(c == 0), stop=(c == NCH - 1))
                        nc.vector.tensor_copy(out=kh_sb[:, m, :], in_=ps_k)
                    # ac scores, one matmul per head: [q, k] -> DRAM bounce [q, bh, k]
                    for h in range(H):
                        ps_ac = psb.tile([Q, S], F32, tag="psac")
                        po = (h % 2) * 64
                        nc.tensor.matmul(ps_ac,
                                         qu_all[po:po + 64, h // 2, b, :],
                                         kh_sb[po:po + 64, h // 2, :], start=True, stop=True)
                        ac_st = phb.tile([Q, S], F16, tag="acst")
                        if h % 2 == 0:
                            nc.vector.tensor_copy(out=ac_st, in_=ps_ac)
                        else:
                            nc.scalar.copy(out=ac_st, in_=ps_ac)
                        nc.sync.dma_start(out=acb_d[:, h * 8 + b, :], in_=ac_st)

                # build block-diagonal qv2 lhsT: col j = h*8+b, rows = head band
                nc.vector.memset(qv2, 0.0)
                for c in range(NCH):
                    for hh in range(2):
                        h = 2 * c + hh
                        for b in range(B):
                            nc.vector.tensor_copy(
                                out=qv2[hh * 64:hh * 64 + 64, c, :, h * 8 + b],
                                in_=qv_all[hh * 64:hh * 64 + 64, c, b, :])

            # ---------------- Pass 1: per-query r / bd / softmax ----------------
            with tc.tile_pool(name="p1", bufs=3) as p1, \
                 tc.tile_pool(name="p1b", bufs=2) as p1b, \
                 tc.tile_pool(name="ps1", bufs=2, space="PSUM") as ps1:
                for q in range(Q):
                    pet = p1.tile([128, NCH, S], F16, tag="pet")
                    nc.sync.dma_start(out=pet, in_=pe_d[q].rearrange("(c p) j -> p c j", p=128))
                    ac_q = p1.tile([64, S], F16, tag="acq")
                    nc.sync.dma_start(out=ac_q, in_=acb_d[q])
                    r16 = p1b.tile([128, NCH, S], F16, tag="r16")
                    for m in range(NCH):
                        ps_r = ps1.tile([128, S], F32, tag="psr")
                        for c in range(NCH):
                            nc.tensor.matmul(ps_r, wr_sb[:, c, m * 128:(m + 1) * 128],
                                             pet[:, c, :], start=(c == 0), stop=(c == NCH - 1))
                        if m % 2 == 0:
                            nc.vector.tensor_copy(out=r16[:, m, :], in_=ps_r)
                        else:
                            nc.scalar.copy(out=r16[:, m, :], in_=ps_r)
                    # bd scores + ac add (extra identity matmul) -> psum [64, 512]
                    ps_bd = ps1.tile([64, S], F32, tag="psbd")
                    for c in range(NCH):
                        nc.tensor.matmul(ps_bd, qv2[:, c, q, :], r16[:, c, :],
                                         start=(c == 0), stop=False)
                    nc.tensor.matmul(ps_bd, ident[:64, :64], ac_q,
                                     start=False, stop=True)
                    # softmax over k (free axis); scale 1/sqrt(DH)=0.125
                    mx = p1b.tile([64, 1], F32, tag="mx")
                    nc.vector.tensor_reduce(out=mx, in_=ps_bd, axis=mybir.AxisListType.X,
                                            op=mybir.AluOpType.max)
                    nm8 = p1b.tile([64, 1], F32, tag="nm8")
                    nc.vector.tensor_scalar_mul(out=nm8, in0=mx, scalar1=-0.125)
                    pexp = p1b.tile([64, S], F32, tag="pexp")
                    rsum = p1b.tile([64, 1], F32, tag="rsum")
                    nc.scalar.activation(out=pexp, in_=ps_bd,
                                         func=mybir.ActivationFunctionType.Exp,
                                         bias=nm8, scale=0.125, accum_out=rsum)
                    rc = p1b.tile([64, 1], F32, tag="rc")
                    nc.vector.reciprocal(out=rc, in_=rsum)
                    p16 = p1b.tile([64, S], F16, tag="p16")
                    nc.scalar.mul(out=p16, in_=pexp, mul=rc)
                    # transpose attn row-block to [k, bh] and stash
                    ps_at = ps1.tile([128, NCH, 64], F16, tag="psat")
                    for c in range(NCH):
                        nc.tensor.transpose(out=ps_at[:, c, :], in_=p16[:, c * 128:(c + 1) * 128],
                                            identity=ident[:64, :64])
                    nc.vector.tensor_copy(out=attn_all[:, :, :, q], in_=ps_at)

            # ---------------- Pass 2: vh / PV / out-proj / LayerNorm ----------------
            with tc.tile_pool(name="p2", bufs=2) as p2, \
                 tc.tile_pool(name="ps2", bufs=2, space="PSUM") as ps2:
                for b in range(B):
                    vt_sb = p2.tile([128, NCH, S], F16, tag="vtl")
                    nc.sync.dma_start(out=vt_sb, in_=vt_d[b].rearrange("(c p) j -> p c j", p=128))
                    vh_sb = p2.tile([128, NCH, D], F16, tag="vh")  # [k_in, kc, hd]
                    for kc in range(NCH):
                        ps_v = ps2.tile([128, D], F32, tag="psv")
                        for c in range(NCH):
                            nc.tensor.matmul(ps_v, vt_sb[:, c, kc * 128:(kc + 1) * 128],
                                             wv_sb[:, c, :], start=(c == 0), stop=(c == NCH - 1))
                        if kc % 2 == 0:
                            nc.vector.tensor_copy(out=vh_sb[:, kc, :], in_=ps_v)
                        else:
                            nc.scalar.copy(out=vh_sb[:, kc, :], in_=ps_v)
                    aot = p2.tile([128, NCH, Q], F16, tag="aot")   # attn_out.T [hd, q]
                    for h in range(H):
                        ps_ao = ps2.tile([64, Q], F32, tag="psao")
                        for c in range(NCH):
                            nc.tensor.matmul(ps_ao, vh_sb[:, c, h * 64:(h + 1) * 64],
                                             attn_all[:, c, h * 8 + b, :],
                                             start=(c == 0), stop=(c == NCH - 1))
                        po = (h % 2) * 64
                        nc.vector.tensor_copy(out=aot[po:po + 64, h // 2, :], in_=ps_ao)
                    ps_o = ps2.tile([Q, D], F32, tag="pso")
                    for c in range(NCH):
                        nc.tensor.matmul(ps_o, aot[:, c, :], wo_sb[:, c, :],
                                         start=(c == 0), stop=(c == NCH - 1))
                    # residual + bo2 + LayerNorm
                    qn_b = p2.tile([Q, D], F32, tag="qnb")
                    nc.sync.dma_start(out=qn_b, in_=qn_d[b])
                    o1 = p2.tile([Q, D], F32, tag="o1")
                    nc.vector.tensor_add(out=o1, in0=ps_o, in1=qn_b)
                    o2 = p2.tile([Q, D], F32, tag="o2")
                    nc.vector.tensor_add(out=o2, in0=o1, in1=bo_sb)
                    st6 = p2.tile([Q, nc.vector.BN_STATS_DIM], F32, tag="st6")
                    nc.vector.bn_stats(out=st6, in_=o2)
                    mv = p2.tile([Q, nc.vector.BN_AGGR_DIM], F32, tag="mv")
                    nc.vector.bn_aggr(out=mv, in_=st6)
                    sd = p2.tile([Q, 1], F32, tag="sd")
                    nc.scalar.activation(out=sd, in_=mv[:, 1:2],
                                         func=mybir.ActivationFunctionType.Sqrt,
                                         bias=eps_sb, scale=1.0)
                    rstd = p2.tile([Q, 1], F32, tag="rstd")
                    nc.vector.reciprocal(out=rstd, in_=sd)
                    mr = p2.tile([Q, 1], F32, tag="mr")
                    nc.vector.tensor_mul(out=mr, in0=mv[:, 0:1], in1=rstd)
                    nmr = p2.tile([Q, 1], F32, tag="nmr")
                    nc.vector.tensor_scalar_mul(out=nmr, in0=mr, scalar1=-1.0)
                    o3 = p2.tile([Q, D], F32, tag="o3")
                    nc.scalar.activation(out=o3, in_=o2,
                                         func=mybir.ActivationFunctionType.Identity,
                                         bias=nmr, scale=rstd)
                    o4 = p2.tile([Q, D], F32, tag="o4")
                    nc.vector.tensor_mul(out=o4, in0=o3, in1=lg_sb)
                    o5 = p2.tile([Q, D], F32, tag="o5")
                    nc.vector.tensor_add(out=o5, in0=o4, in1=lb_sb)
                    nc.sync.dma_start(out=out_d[b], in_=o5)

    nc.compile()
    return nc


def _toeplitz_table(pos):
    """Extract the 1023-row relative-position table if pos_emb has the
    Transformer-XL structure pos[q, k] == table[k - q + 511]; else None."""
    if pos.shape != (S, S, D):
        return None
    table = np.concatenate([pos[S - 1:0:-1, 0, :], pos[0, :, :]], axis=0)
    rng = np.random.default_rng(12345)
    qs = rng.integers(0, S, 96)
    ks = rng.integers(0, S, 96)
    for q, k in zip(qs, ks):
        if not np.array_equal(pos[q, k], table[k - q + S - 1]):
            return None
    return table


def kernel(**inputs):
    global last_result
    f16, f32 = np.float16, np.float32
    q = np.asarray(inputs["q"], f32)
    k = np.asarray(inputs["k"], f32)
    v = np.asarray(inputs["v"], f32)
    pos = np.asarray(inputs["pos_emb"], f32)
    Wq, Wk, Wv, Wr, Wo = (np.asarray(inputs[n], f32) for n in ("Wq", "Wk", "Wv", "Wr", "Wo"))
    bq, bo, bvb = (np.asarray(inputs[n], f32) for n in ("bq", "bo", "bv"))
    u_b = np.asarray(inputs["u_bias"], f32).reshape(-1)
    v_b = np.asarray(inputs["v_bias"], f32).reshape(-1)
    lng, lnb = np.asarray(inputs["ln_g"], f32), np.asarray(inputs["ln_b"], f32)

    bo2 = (bo + Wo @ bvb).astype(f32)

    table = _toeplitz_table(pos)
    if table is not None:
        if "nc2d" not in _CACHE:
            _CACHE["nc2d"] = _build_2d()
        nc = _CACHE["nc2d"]

        # shared staging
        tbl_pad = np.zeros((JW, D), f32)
        tbl_pad[:2 * S - 1] = table
        pet = np.ascontiguousarray(
            tbl_pad.T.reshape(NCH, 128, JW).transpose(1, 0, 2)).astype(f16)
        bu_full = (bq + u_b)
        bv_full = (bq + v_b)
        wqt, wkt, wvt, wrt = (np.ascontiguousarray(W.T) for W in (Wq, Wk, Wv, Wr))
        wot = np.ascontiguousarray(Wo.T)
        lg_s = lng.reshape(1, D).astype(f32)
        lb_s = lnb.reshape(1, D).astype(f32)

        def dmajor(x):  # [BL, S, D] -> [128, NCH, BL, S]
            return np.ascontiguousarray(
                x.transpose(0, 2, 1).reshape(BL, NCH, 128, S).transpose(2, 1, 0, 3)
            ).astype(f16)

        def wpair(wt, hd):  # W.T[:, hd] -> [128, NCH, 128]
            return np.ascontiguousarray(
                wt[:, hd].reshape(NCH, 128, 128).transpose(1, 0, 2)).astype(f16)

        in_maps = []
        for c in range(NCORES):
            bg, hg = c // 4, c % 4
            bs = slice(bg * BL, (bg + 1) * BL)
            hd = slice(hg * 128, (hg + 1) * 128)
            in_maps.append(dict(
                qt=dmajor(q[bs]), kt=dmajor(k[bs]), vt=dmajor(v[bs]),
                pet=pet,
                wqp=wpair(wqt, hd), wkp=wpair(wkt, hd),
                wvp=wpair(wvt, hd), wrp=wpair(wrt, hd),
                wop=np.ascontiguousarray(wot[hd, :]).astype(f16),
                bu=np.ascontiguousarray(bu_full[hd]).reshape(128, 1).astype(f32),
                bv2=np.ascontiguousarray(bv_full[hd]).reshape(128, 1).astype(f32),
                qn=np.ascontiguousarray(q[bs, hd, :] + bo2).astype(f32),
                lng=lg_s, lnb=lb_s,
            ))
        res = run_bass_kernel_spmd(nc, in_maps, core_ids=list(range(NCORES)))
        last_result = res
        out = np.empty((B, S, D), f32)
        for c in range(NCORES):
            bg, hg = c // 4, c % 4
            out[bg * BL:(bg + 1) * BL, hg * 128:(hg + 1) * 128, :] = res.results[c]["out"]
        return out

    # ---------------- fallback: arbitrary pos_emb ----------------
    if "nc" not in _CACHE:
        _CACHE["nc"] = _build()
    nc = _CACHE["nc"]
    wqt = np.ascontiguousarray(Wq.T).astype(f16)
    wkt = np.ascontiguousarray(Wk.T).astype(f16)
    wvt = np.ascontiguousarray(Wv.T).astype(f16)
    wrt = np.ascontiguousarray(Wr.T).astype(f16)
    wot = np.ascontiguousarray(Wo.T).astype(f16)
    bu = np.ascontiguousarray((bq + u_b).reshape(NCH, 128).T).astype(f32)
    bv2 = np.ascontiguousarray((bq + v_b).reshape(NCH, 128).T).astype(f32)
    kt = np.ascontiguousarray(k.transpose(0, 2, 1)).astype(f16)
    vt = np.ascontiguousarray(v.transpose(0, 2, 1)).astype(f16)
    qt_full = np.ascontiguousarray(q.transpose(0, 2, 1)).astype(f16)   # [B, D, S]
    shared = dict(kt=kt, vt=vt, wqt=wqt, wkt=wkt, wvt=wvt, wrt=wrt, wot=wot,
                  bu=bu, bv2=bv2, bo2=bo2.reshape(1, D),
                  lng=lng.reshape(1, D).astype(f32), lnb=lnb.reshape(1, D).astype(f32))
    in_maps = []
    pos_t = pos.transpose(0, 2, 1)                                 # view [q, D, k]
    for c in range(NCORES):
        sl = slice(c * Q, (c + 1) * Q)
        qt_c = qt_full[:, :, sl].reshape(B, NCH, 128, Q).transpose(2, 1, 0, 3)
        in_maps.append(dict(shared,
                            pe=np.ascontiguousarray(pos_t[sl]).astype(f16),
                            qt=np.ascontiguousarray(qt_c),
                            qn=np.ascontiguousarray(q[:, sl, :])))

    res = run_bass_kernel_spmd(nc, in_maps, core_ids=list(range(NCORES)))
    last_result = res
    out = np.concatenate([r["out"] for r in res.results], axis=1)
    return out.astype(f32)
